# revision 1
# baseline (speedup 1.0000x reference)
"""Mamba-enhance kernel for Trainium2, data-parallel over batch across 8 NeuronCores.

Self-contained: takes the FULL inputs of nn_Enhance_26319559590732, shards the
batch (8) across 8 cores, runs a Bass/Tile kernel per core, gathers the output.

Per-core layout: channel-on-partition [d, l] throughout (l = H*W = 4096).
Selective scan via DVE tensor_tensor_scan per (state n, d-half) plane; the
sum over n of C*h is accumulated on the TensorEngine with an identity matmul.
B/C rows are broadcast across partitions by the DMA engines (stride-0 DRAM
reads), keeping GPSIMD free.
"""

import functools
import os

import ml_dtypes
import numpy as np

import concourse.bass as bass
import concourse.tile as tile
from concourse import bacc, mybir
from concourse.bass_utils import run_bass_kernel_spmd

F32 = mybir.dt.float32
BF16 = mybir.dt.bfloat16
AF = mybir.ActivationFunctionType
ALU = mybir.AluOpType

B = 8
D_MODEL = 128
D_STATE = 16
D_INNER = 256
DT_RANK = 8
GROUPS = 4
EPS = 1e-5
L = 64 * 64  # 4096
T = 2048  # DVE chunk
NCHUNK = L // T
NSUB = T // 512  # 512-wide psum subchunks per T-chunk


def _bf(x):
    return np.ascontiguousarray(np.asarray(x).astype(ml_dtypes.bfloat16))


def _f(x):
    return np.ascontiguousarray(np.asarray(x).astype(np.float32))


@functools.lru_cache(maxsize=4)
def _build(fir_states=(), k1_states=(), gps_mod=5):
    nc = bacc.Bacc("TRN2", target_bir_lowering=False, debug=False, num_devices=B)

    # ---- DRAM I/O ----
    x_f_d = nc.dram_tensor("x_f", [128, L], F32, kind="ExternalInput")
    x_bf_d = nc.dram_tensor("x_bf", [128, L], BF16, kind="ExternalInput")
    w_in_d = nc.dram_tensor("w_in", [128, 512], BF16, kind="ExternalInput")
    # [kh, 128, M] layouts for K=256 weights
    w_x_d = nc.dram_tensor("w_x", [2, 128, 40], BF16, kind="ExternalInput")
    w_eff_d = nc.dram_tensor("w_eff", [2, 128, 256], BF16, kind="ExternalInput")
    w_out_d = nc.dram_tensor("w_out", [2, 128, 128], BF16, kind="ExternalInput")
    # per-partition vectors, [128, 2] = (d_half)
    conv_w0_d = nc.dram_tensor("conv_w0", [128, 2], F32, kind="ExternalInput")
    conv_w1_d = nc.dram_tensor("conv_w1", [128, 2], F32, kind="ExternalInput")
    conv_b_d = nc.dram_tensor("conv_b", [128, 2], F32, kind="ExternalInput")
    b_dt_d = nc.dram_tensor("b_dt", [128, 2], F32, kind="ExternalInput")
    dvec_d = nc.dram_tensor("dvec", [128, 2], F32, kind="ExternalInput")
    a_sc_d = nc.dram_tensor("a_sc", [128, 2, D_STATE], F32, kind="ExternalInput")
    ident_d = nc.dram_tensor("ident", [128, 128], BF16, kind="ExternalInput")
    cbsel_d = nc.dram_tensor("cbsel", [16, 1], BF16, kind="ExternalInput")
    gmat_d = nc.dram_tensor("gmat", [128, GROUPS], BF16, kind="ExternalInput")
    g2_d = nc.dram_tensor("g2", [4, 128], F32, kind="ExternalInput")
    gam_d = nc.dram_tensor("gam", [128, 1], F32, kind="ExternalInput")
    bet_d = nc.dram_tensor("bet", [128, 1], F32, kind="ExternalInput")

    out_d = nc.dram_tensor("out", [128, L], F32, kind="ExternalOutput")
    gnscratch = nc.dram_tensor("gnscratch", [8], F32)
    bcrows_d = nc.dram_tensor("bcrows", [40, L], BF16)  # x_dbl rows, DMA-bcast source
    cbrows_d = nc.dram_tensor("cbrows", [1, L], BF16)   # combined tap-0 row: sum CB over k1+fir
    cb1rows_d = nc.dram_tensor("cb1rows", [16, L], BF16)  # C[t]*B[t-1] rows for FIR tap-1

    with tile.TileContext(nc) as tc:
        with (
            tc.tile_pool(name="persist", bufs=1) as pp,
            tc.tile_pool(name="scratch", bufs=2) as ss,
            tc.tile_pool(name="psum", bufs=8, space="PSUM") as ps,
        ):
            # ---- load constants/weights ----
            w_in = pp.tile([128, 512], BF16)
            w_x = pp.tile([128, 2, 40], BF16)
            w_eff = pp.tile([128, 2, 256], BF16)
            w_out = pp.tile([128, 2, 128], BF16)
            conv_w0 = pp.tile([128, 2], F32)
            conv_w1 = pp.tile([128, 2], F32)
            conv_b = pp.tile([128, 2], F32)
            b_dt = pp.tile([128, 2], F32)
            dvec = pp.tile([128, 2], F32)
            a_sc = pp.tile([128, 2, D_STATE], F32)
            ident = pp.tile([128, 128], BF16)
            cbsel = pp.tile([16, 1], BF16)
            gmat = pp.tile([128, GROUPS], BF16)
            g2 = pp.tile([4, 128], F32)
            gam = pp.tile([128, 1], F32)
            bet = pp.tile([128, 1], F32)

            nc.sync.dma_start(w_in[:], w_in_d[:])
            nc.sync.dma_start(w_x[:], w_x_d[:].rearrange("h p m -> p h m"))
            nc.sync.dma_start(w_eff[:], w_eff_d[:].rearrange("h p m -> p h m"))
            nc.sync.dma_start(w_out[:], w_out_d[:].rearrange("h p m -> p h m"))
            nc.sync.dma_start(conv_w0[:], conv_w0_d[:])
            nc.sync.dma_start(conv_w1[:], conv_w1_d[:])
            nc.sync.dma_start(conv_b[:], conv_b_d[:])
            nc.sync.dma_start(b_dt[:], b_dt_d[:])
            nc.sync.dma_start(dvec[:], dvec_d[:])
            nc.sync.dma_start(a_sc[:], a_sc_d[:])
            nc.sync.dma_start(ident[:], ident_d[:])
            nc.sync.dma_start(cbsel[:], cbsel_d[:])
            nc.sync.dma_start(gmat[:], gmat_d[:])
            nc.sync.dma_start(g2[:], g2_d[:])
            nc.sync.dma_start(gam[:], gam_d[:])
            nc.sync.dma_start(bet[:], bet_d[:])

            # ---- persistent activations ----
            xh_bf = pp.tile([128, 2, L], BF16)   # conv+silu output
            z_bf = pp.tile([128, 2, L], BF16)    # silu(z) gate
            dt_f = pp.tile([128, 2, L], F32)     # softplus dt
            y2_bf = pp.tile([128, 2, T], BF16)   # gated y, per chunk
            out_pre = pp.tile([128, L], BF16)    # pre-groupnorm out
            hlast = pp.tile([128, 32], F32)      # scan carry per (h, n)
            sb_st = pp.tile([GROUPS, 2, 512], F32)  # groupnorm sum/sumsq accumulators
            dlast = pp.tile([128, 32], BF16)     # last dbx col per (h, n) for FIR

            # ================= Phase A: in_proj, conv, silu =================
            x_bf = ss.tile([128, L], BF16, tag="sc8a", bufs=1)
            nc.sync.dma_start(x_bf[:], x_bf_d[:])
            for h in range(2):
                xh_f = ss.tile([128, L], BF16, tag="xhf", bufs=1)
                # xz block m=h -> xh_pre half h ; block m=2+h -> z half h
                for m in (h, 2 + h):
                    for c in range(L // 512):
                        mm = ps.tile([128, 512], F32, tag="bank", name=f"inp_{m}_{c}")
                        nc.tensor.matmul(
                            mm[:], w_in[:, bass.ts(m, 128)], x_bf[:, bass.ts(c, 512)],
                            start=True, stop=True,
                        )
                        if m < 2:
                            nc.vector.tensor_copy(xh_f[:, bass.ts(c, 512)], mm[:])
                        else:
                            nc.scalar.activation(
                                z_bf[:, m - 2, bass.ts(c, 512)], mm[:], AF.Silu,
                            )
                # causal depthwise conv k=2 + silu (chunked; shifts stay inside xh_f)
                for c in range(NCHUNK):
                    t1 = ss.tile([128, T], F32, tag="f4a", bufs=2)
                    nc.vector.tensor_scalar_mul(
                        t1[:], xh_f[:, bass.ts(c, T)], conv_w1[:, h:h + 1]
                    )
                    cv = ss.tile([128, T], F32, tag="f4b", bufs=1)
                    if c == 0:
                        nc.vector.scalar_tensor_tensor(
                            cv[:, 1:T], xh_f[:, 0:T - 1], conv_w0[:, h:h + 1],
                            t1[:, 1:T], ALU.mult, ALU.add,
                        )
                        nc.vector.tensor_copy(cv[:, 0:1], t1[:, 0:1])
                    else:
                        nc.vector.scalar_tensor_tensor(
                            cv[:], xh_f[:, c * T - 1:(c + 1) * T - 1],
                            conv_w0[:, h:h + 1], t1[:], ALU.mult, ALU.add,
                        )
                    nc.scalar.activation(
                        xh_bf[:, h, bass.ts(c, T)], cv[:], AF.Silu,
                        bias=conv_b[:, h:h + 1],
                    )

            # ================= Phase B: x_proj, dt =================
            bc_rows = ss.tile([40, L], BF16, tag="sc8a", bufs=1)
            for c in range(L // 512):
                mm = ps.tile([128, 512], F32, tag="bank", name=f"xdbl_{c}")
                for kh in range(2):
                    nc.tensor.matmul(
                        mm[0:40, :], w_x[:, kh, :], xh_bf[:, kh, bass.ts(c, 512)],
                        start=(kh == 0), stop=(kh == 1),
                    )
                nc.vector.tensor_copy(bc_rows[:, bass.ts(c, 512)], mm[0:40, :])
            # stage B/C rows to DRAM so DMA engines can partition-broadcast them
            nc.sync.dma_start(bcrows_d[:], bc_rows[:])
            for dh in range(2):
                for cc in range(NCHUNK):
                    dte = ss.tile([128, T], F32, tag="f4b", bufs=1,
                                  name=f"dte_{dh}_{cc}")
                    for q in range(T // 512):
                        cq = cc * (T // 512) + q
                        mm = ps.tile([128, 512], F32, tag="bank", name=f"dtp_{dh}_{cq}")
                        for kh in range(2):
                            nc.tensor.matmul(
                                mm[:], w_eff[:, kh, bass.ts(dh, 128)],
                                xh_bf[:, kh, bass.ts(cq, 512)],
                                start=(kh == 0), stop=(kh == 1),
                            )
                        # softplus(v) = ln(1+exp(v)); one ACT table set for both
                        nc.scalar.activation(
                            dte[:, bass.ts(q, 512)], mm[:], AF.Exp,
                            bias=b_dt[:, dh:dh + 1],
                        )
                    nc.scalar.activation(
                        dt_f[:, dh, bass.ts(cc, T)], dte[:], AF.Ln, bias=1.0,
                    )

            # ================= Phase C: selective scan =================
            for c in range(NCHUNK):
                # dtx for this chunk (reused by all 16 states)
                dtx = [None, None]
                for h in range(2):
                    dtx[h] = ss.tile([128, T], BF16, tag=f"dtx{h}", name=f"dtx_{c}_{h}")
                    nc.vector.tensor_tensor(
                        dtx[h][:], dt_f[:, h, bass.ts(c, T)],
                        xh_bf[:, h, bass.ts(c, T)], ALU.mult,
                    )
                if k1_states or fir_states:
                    cbb = ss.tile([16, T], BF16, tag="b_bc", name=f"cbb_{c}")
                    cbc = ss.tile([16, T], BF16, tag="c_bc", name=f"cbc_{c}")
                    nc.sync.dma_start(
                        cbb[:],
                        bass.AP(tensor=bcrows_d[:].tensor, offset=8 * L + c * T,
                                ap=[[L, 16], [1, T]]),
                    )
                    nc.sync.dma_start(
                        cbc[:],
                        bass.AP(tensor=bcrows_d[:].tensor, offset=24 * L + c * T,
                                ap=[[L, 16], [1, T]]),
                    )
                    # sum of C*B rows over all tap-0-absorbed states (exact, linear)
                    cbs = ss.tile([16, T], BF16, tag="y1", bufs=1, name=f"cbs_{c}")
                    nc.vector.tensor_tensor(cbs[:], cbb[:], cbc[:], ALU.mult)
                    crow = ss.tile([1, T], BF16, tag="y1", bufs=1, name=f"crow_{c}")
                    for q in range(T // 512):
                        csum_ps = ps.tile([1, 512], F32, tag="bank",
                                          name=f"csum_ps_{c}_{q}")
                        nc.tensor.matmul(
                            csum_ps[:], cbsel[:], cbs[:, bass.ts(q, 512)],
                            start=True, stop=True,
                        )
                        nc.scalar.copy(crow[:, bass.ts(q, 512)], csum_ps[:])
                    nc.sync.dma_start(cbrows_d[:, bass.ts(c, T)], crow[:])
                    if fir_states:
                        # CB1[t] = C[t] * B[t-1] rows (col 0 garbage; masked by w[:,0]=0)
                        cbbs = ss.tile([16, T], BF16, tag="b_bc", name=f"cbbs_{c}")
                        nc.sync.dma_start(
                            cbbs[:],
                            bass.AP(tensor=bcrows_d[:].tensor,
                                    offset=8 * L + c * T - 1, ap=[[L, 16], [1, T]]),
                        )
                        cbs1 = ss.tile([16, T], BF16, tag="y1", bufs=1, name=f"cbs1_{c}")
                        nc.vector.tensor_tensor(cbs1[:], cbc[:], cbbs[:], ALU.mult)
                        nc.sync.dma_start(cb1rows_d[:, bass.ts(c, T)], cbs1[:])
                ysub = [
                    ps.tile([128, 512], F32, tag="bank", name=f"ysub_{c}_{i}")
                    for i in range(2 * NSUB)
                ]
                # accumulation order: combined tap-0 plane, scan states, FIR tap-1
                seq = []
                if k1_states or fir_states:
                    seq.append(("tap0", -1))
                scan_ns = [n for n in range(D_STATE)
                           if not all((h, n) in (set(k1_states) | set(fir_states))
                                      for h in range(2))]
                fir_ns = sorted({n for (h, n) in fir_states})
                seq += [("scan", n) for n in scan_ns]
                seq += [("fir", n) for n in fir_ns]
                for si, (kind, n) in enumerate(seq):
                    first, last = si == 0, si == len(seq) - 1
                    if kind == "tap0":
                        cb_bc = ss.tile([128, T], BF16, tag="b_bc",
                                        name=f"cbbc_{c}")
                        nc.sync.dma_start(
                            cb_bc[:],
                            bass.AP(tensor=cbrows_d[:].tensor,
                                    offset=c * T, ap=[[0, 128], [1, T]]),
                        )
                        for h in range(2):
                            hc = ss.tile([128, T], BF16, tag="hc",
                                         name=f"hck_{c}_{h}")
                            nc.vector.tensor_tensor(
                                hc[:], dtx[h][:], cb_bc[:], ALU.mult,
                            )
                            for s in range(NSUB):
                                nc.tensor.matmul(
                                    ysub[h * NSUB + s][:], ident[:],
                                    hc[:, bass.ts(s, 512)],
                                    start=first, stop=last,
                                )
                        continue
                    if kind == "fir":
                        # tap-1 only: y_n += (da*dtx[t-1]) * CB1
                        cb1_bc = ss.tile([128, T], BF16, tag="c_bc",
                                         name=f"cb1bc_{c}_{n}")
                        nc.sync.dma_start(
                            cb1_bc[:],
                            bass.AP(tensor=cb1rows_d[:].tensor,
                                    offset=n * L + c * T, ap=[[0, 128], [1, T]]),
                        )
                        for h in range(2):
                            da = ss.tile([128, T], BF16, tag="dab", bufs=2,
                                         name=f"daf_{c}_{n}_{h}")
                            nc.scalar.activation(
                                da[:], dt_f[:, h, bass.ts(c, T)], AF.Exp,
                                scale=a_sc[:, h, n:n + 1],
                            )
                            w = ss.tile([128, T], BF16, tag="dbx",
                                        name=f"w_{c}_{n}_{h}")
                            nc.vector.tensor_tensor(
                                w[:, 1:T], da[:, 1:T], dtx[h][:, 0:T - 1], ALU.mult,
                            )
                            if c == 0:
                                nc.vector.memset(w[:, 0:1], 0.0)
                            else:
                                nc.vector.tensor_tensor(
                                    w[:, 0:1], da[:, 0:1],
                                    dlast[:, h * 16 + n:h * 16 + n + 1], ALU.mult,
                                )
                            if c < NCHUNK - 1:
                                nc.vector.tensor_copy(
                                    dlast[:, h * 16 + n:h * 16 + n + 1],
                                    dtx[h][:, T - 1:T],
                                )
                            hc = ss.tile([128, T], BF16, tag="hc",
                                         name=f"hcf_{c}_{n}_{h}")
                            nc.vector.tensor_tensor(hc[:], w[:], cb1_bc[:], ALU.mult)
                            for s in range(NSUB):
                                nc.tensor.matmul(
                                    ysub[h * NSUB + s][:], ident[:],
                                    hc[:, bass.ts(s, 512)],
                                    start=first, stop=last,
                                )
                        continue
                    # exact scan state
                    b_bc = ss.tile([128, T], BF16, tag="b_bc",
                                   name=f"bbc_{c}_{n}")
                    c_bc = ss.tile([128, T], BF16, tag="c_bc",
                                   name=f"cbc2_{c}_{n}")
                    nc.sync.dma_start(
                        b_bc[:],
                        bass.AP(tensor=bcrows_d[:].tensor,
                                offset=(8 + n) * L + c * T, ap=[[0, 128], [1, T]]),
                    )
                    nc.sync.dma_start(
                        c_bc[:],
                        bass.AP(tensor=bcrows_d[:].tensor,
                                offset=(24 + n) * L + c * T, ap=[[0, 128], [1, T]]),
                    )
                    for h in range(2):
                        da = ss.tile([128, T], F32, tag="f4a", bufs=2,
                                     name=f"da_{c}_{n}_{h}")
                        nc.scalar.activation(
                            da[:], dt_f[:, h, bass.ts(c, T)], AF.Exp,
                            scale=a_sc[:, h, n:n + 1],
                        )
                        dbx = ss.tile([128, T], BF16, tag="dbx",
                                      name=f"dbx_{c}_{n}_{h}")
                        nc.vector.tensor_tensor(
                            dbx[:], dtx[h][:], b_bc[:], ALU.mult,
                        )
                        ht = ss.tile([128, T], BF16, tag="ht",
                                     name=f"ht_{c}_{n}_{h}")
                        ini = 0.0 if c == 0 else hlast[:, h * 16 + n:h * 16 + n + 1]
                        nc.vector.tensor_tensor_scan(
                            ht[:], da[:], dbx[:], ini, ALU.mult, ALU.add,
                        )
                        if c < NCHUNK - 1:
                            nc.vector.tensor_copy(
                                hlast[:, h * 16 + n:h * 16 + n + 1], ht[:, T - 1:T],
                            )
                        hc = ss.tile([128, T], BF16, tag="hc",
                                     name=f"hc_{c}_{n}_{h}")
                        nc.vector.tensor_tensor(hc[:], ht[:], c_bc[:], ALU.mult)
                        for s in range(NSUB):
                            nc.tensor.matmul(
                                ysub[h * NSUB + s][:], ident[:], hc[:, bass.ts(s, 512)],
                                start=first, stop=last,
                            )
                # gating: y2 = (y + xh*D) * silu(z)
                for h in range(2):
                    for s in range(NSUB):
                        col = c * T + s * 512
                        y1 = ss.tile([128, 512], BF16, tag="y1", bufs=1)
                        nc.vector.scalar_tensor_tensor(
                            y1[:], xh_bf[:, h, col:col + 512], dvec[:, h:h + 1],
                            ysub[h * NSUB + s][:], ALU.mult, ALU.add,
                        )
                        nc.vector.tensor_tensor(
                            y2_bf[:, h, bass.ts(s, 512)], y1[:],
                            z_bf[:, h, col:col + 512], ALU.mult,
                        )
                # out_proj for this chunk
                stp_s = ps.tile([GROUPS, 512], F32, tag="bank", name=f"stps_{c}")
                stp_q = ps.tile([GROUPS, 512], F32, tag="bank", name=f"stpq_{c}")
                for s in range(NSUB):
                    mo = ps.tile([128, 512], F32, tag="bank", name=f"oproj_{c}_{s}")
                    for kh in range(2):
                        nc.tensor.matmul(
                            mo[:], w_out[:, kh, :], y2_bf[:, kh, bass.ts(s, 512)],
                            start=(kh == 0), stop=(kh == 1),
                        )
                    nc.scalar.copy(out_pre[:, c * T + s * 512:c * T + (s + 1) * 512], mo[:])
                    gs = c * NSUB + s
                    sqs = ss.tile([128, 512], BF16, tag="y1", bufs=1, name=f"sqs_{gs}")
                    nc.scalar.activation(
                        sqs[:], out_pre[:, c * T + s * 512:c * T + (s + 1) * 512],
                        AF.Square,
                    )
                    nc.tensor.matmul(
                        stp_s[:], gmat[:],
                        out_pre[:, c * T + s * 512:c * T + (s + 1) * 512],
                        start=(s == 0), stop=(s == NSUB - 1),
                    )
                    nc.tensor.matmul(
                        stp_q[:], gmat[:], sqs[:],
                        start=(s == 0), stop=(s == NSUB - 1),
                    )
                # fold this chunk's group stats into the SBUF accumulators
                if c == 0:
                    nc.vector.tensor_copy(sb_st[:, 0, :], stp_s[:])
                    nc.vector.tensor_copy(sb_st[:, 1, :], stp_q[:])
                else:
                    nc.vector.tensor_tensor(sb_st[:, 0, :], sb_st[:, 0, :], stp_s[:], ALU.add)
                    nc.vector.tensor_tensor(sb_st[:, 1, :], sb_st[:, 1, :], stp_q[:], ALU.add)

            # ================= Phase D: groupnorm + silu + residual =================
            red = pp.tile([GROUPS, 2], F32)
            nc.vector.tensor_reduce(
                red[:, 0:1], sb_st[:, 0, :], mybir.AxisListType.X, ALU.add
            )
            nc.vector.tensor_reduce(
                red[:, 1:2], sb_st[:, 1, :], mybir.AxisListType.X, ALU.add
            )
            # mean = s/N ; var = q/N - mean^2 ; rstd = 1/sqrt(var+eps)
            NG = float(32 * L)
            mv = pp.tile([GROUPS, 4], F32)
            nc.scalar.mul(mv[:, 0:1], red[:, 0:1], 1.0 / NG)   # mean
            nc.scalar.mul(mv[:, 1:2], red[:, 1:2], 1.0 / NG)   # E[x^2]
            msq = pp.tile([GROUPS, 1], F32)
            nc.vector.tensor_tensor(msq[:], mv[:, 0:1], mv[:, 0:1], ALU.mult)
            nc.vector.tensor_tensor(mv[:, 2:3], mv[:, 1:2], msq[:], ALU.subtract)  # var
            epst = pp.tile([GROUPS, 1], F32)
            nc.vector.memset(epst[:], EPS)
            nc.scalar.activation(mv[:, 3:4], mv[:, 2:3], AF.Sqrt, bias=epst[:])
            nc.vector.reciprocal(mv[:, 3:4], mv[:, 3:4])       # rstd
            # replicate group stats to 128 channels with one tiny PE matmul
            mpick = pp.tile([GROUPS, 2], F32)
            nc.vector.tensor_copy(mpick[:, 0:1], mv[:, 0:1])
            nc.vector.tensor_copy(mpick[:, 1:2], mv[:, 3:4])
            mr_ps = ps.tile([128, 2], F32, tag="bank", name="mr_ps")
            nc.tensor.matmul(mr_ps[:], g2[:], mpick[:], start=True, stop=True)
            scale_pp = pp.tile([128, 1], F32)
            bias_pp = pp.tile([128, 1], F32)
            nc.vector.tensor_tensor(scale_pp[:], gam[:], mr_ps[:, 1:2], ALU.mult)
            tmp = pp.tile([128, 1], F32)
            nc.vector.tensor_tensor(tmp[:], mr_ps[:, 0:1], scale_pp[:], ALU.mult)
            nc.vector.tensor_tensor(bias_pp[:], bet[:], tmp[:], ALU.subtract)
            # final: silu(out_pre*scale + bias) + x
            for c in range(NCHUNK):
                x_re = ss.tile([128, T], F32, tag="f4c", bufs=1)
                nc.sync.dma_start(x_re[:], x_f_d[:, bass.ts(c, T)])
                fin = ss.tile([128, T], F32, tag="f4b", bufs=1)
                nc.scalar.activation(
                    fin[:], out_pre[:, bass.ts(c, T)], AF.Silu,
                    scale=scale_pp[:], bias=bias_pp[:],
                )
                fo = ss.tile([128, T], F32, tag="f4a", bufs=2)
                nc.vector.tensor_tensor(fo[:], fin[:], x_re[:], ALU.add)
                nc.sync.dma_start(out_d[:, bass.ts(c, T)], fo[:])

    nc.compile()
    return nc


def _prep_weights(W_in, conv_w, conv_b, W_x, W_dt, b_dt, A_log, D, W_out, gn_gamma, gn_beta):
    W_eff = _f(W_x)[:, :DT_RANK] @ _f(W_dt)  # [256, 256]
    A = -np.exp(_f(A_log))  # [256, 16]
    half = lambda v: np.stack([_f(v)[:128], _f(v)[128:]], axis=1)  # [128, 2]
    ident = np.eye(128, dtype=np.float32)
    gmat = np.zeros((128, GROUPS), np.float32)
    for g in range(GROUPS):
        gmat[g * 32:(g + 1) * 32, g] = 1.0
    W_x, W_out, conv_w = _f(W_x), _f(W_out), _f(conv_w)
    return {
        "w_in": _bf(_f(W_in)),
        "w_x": _bf(np.stack([W_x[:128, :], W_x[128:, :]])),
        "w_eff": _bf(np.stack([W_eff[:128, :], W_eff[128:, :]])),
        "w_out": _bf(np.stack([W_out[:128, :], W_out[128:, :]])),
        "conv_w0": half(conv_w[:, 0]),
        "conv_w1": half(conv_w[:, 1]),
        "conv_b": half(conv_b),
        "b_dt": half(b_dt),
        "dvec": half(D),
        "a_sc": _f(np.stack([A[:128, :], A[128:, :]], axis=1)),  # [128, 2, 16]
        "ident": _bf(ident),
        "gmat": _bf(gmat),
        "g2": _f(gmat.T),
        "gam": _f(gn_gamma).reshape(128, 1),
        "bet": _f(gn_beta).reshape(128, 1),
    }


def kernel(x_hsi, W_in, conv_w, conv_b, W_x, W_dt, b_dt, A_log, D, W_out, gn_gamma, gn_beta):
    # states whose decay is fast enough that a 2-tap FIR is exact to ~1e-4:
    # per-step log-decay >= |A|*dt_min; 2 taps -> error exp(-2*|A|*dt_min)
    bmin = float(_f(b_dt).min())
    dt_min = float(np.log1p(np.exp(bmin - 0.2)))  # softplus with data margin
    A_abs = np.exp(_f(A_log))  # [256, 16]
    fir, k1 = [], []
    for h in range(2):
        amin = A_abs[h * 128:(h + 1) * 128, :].min(axis=0)  # [16]
        for n in range(D_STATE):
            if float(amin[n]) * dt_min >= 4.3:
                k1.append((h, n))
            elif 2.0 * float(amin[n]) * dt_min >= 3.8:
                fir.append((h, n))
    nc = _build(tuple(sorted(fir)), tuple(sorted(k1)),
                int(os.environ.get("BASS_GPS_MOD", "5")))
    cbsel = np.zeros((16, 1), np.float32)
    for (h, n) in set(k1) | set(fir):
        cbsel[n, 0] = 1.0
    wmap = _prep_weights(W_in, conv_w, conv_b, W_x, W_dt, b_dt, A_log, D, W_out, gn_gamma, gn_beta)
    in_maps = []
    for b in range(B):
        xc = _f(x_hsi[b]).reshape(128, L)
        m = dict(wmap)
        m["x_f"] = xc
        m["x_bf"] = _bf(xc)
        m["cbsel"] = _bf(cbsel)
        in_maps.append(m)
    trace = bool(int(os.environ.get("BASS_KERNEL_TRACE", "0")))
    res = run_bass_kernel_spmd(nc, in_maps, list(range(B)), trace=trace)
    if trace:
        kernel.last_exec_time_ns = res.exec_time_ns
        kernel.last_insts = res.instructions_and_trace
    out = np.stack([res.results[b]["out"].reshape(D_MODEL, 64, 64) for b in range(B)])
    return out.astype(np.float32)

